# revision 94
# baseline (speedup 1.0000x reference)
"""Distributed GQA attention block (dense_transformer) for 8 TRN2 NeuronCores.

Reference computation (all fp32):
    q = (x @ wq)  -> RoPE;  k = (x @ wk) -> RoPE;  v = x @ wv
    causal softmax(q k^T / sqrt(64)) @ v  (GQA: 32 q heads, 4 kv heads)
    out = attn_out @ wo

Sharding: core (b, g) for b in {0,1}, g in {0..3} handles batch b, q-heads
8g..8g+7, kv-head g (data-parallel over batch x tensor-parallel over GQA
groups).  Each core computes attn_out for its heads ([512, 2048],
feature-major) and a full-width partial output
partial = wo[512g:512g+512, :]^T @ attn_out  ([2048, T]); a ReduceScatter
(add) per 512-token block sums the partials within the 4-core batch group
and leaves each core with its disjoint 512-row slice of the final output.
Attention runs token-block-outer (qb) so each block's partial + RS is
pipelined behind the later blocks' compute; no AllGather, attn_out never
leaves SBUF.

All activations/weights are kept feature-major (transposed) on chip so every
matmul contracts over the partition dim with no on-chip transposes except a
single small one for v.  Matmul compute in bf16 (fp32 PSUM accumulate).
"""

import json

import numpy as np
import ml_dtypes

import concourse.bass as bass
import concourse.bass2jax as bass2jax
import concourse.mybir as mybir
import concourse.tile as tile
from concourse.tile import VectorClock, ScopedClock
from concourse.bass_utils import compile_bir_kernel, run_bass_kernel_spmd

_MAX_WAITS = 1  # this walrus build rejects instructions with more sem waits


def _split_excess_waits(bir_json, max_waits=_MAX_WAITS):
    """Hoist excess per-instruction sem waits onto injected same-engine NoOps.

    The TRN2 ISA encoding in this neuronxcc build allows at most `max_waits`
    sync-wait commands per instruction; Tile's sem assigner can emit more.
    A NoOp inserted immediately before the instruction on the same engine is
    semantically identical (the engine blocks at the same program point).
    """
    d = json.loads(bir_json)
    changed = False
    for fn in d.get("functions", []):
        for bb in fn.get("blocks", []):
            insts = bb.get("instructions", [])
            new = []
            for ins in insts:
                si = ins.get("sync_info")
                waits = (si or {}).get("on_wait") or []
                if len(waits) > max_waits:
                    changed = True
                    excess, keep = waits[:-max_waits], waits[-max_waits:]
                    for i in range(0, len(excess), max_waits):
                        new.append(
                            {
                                "debug": ins.get("debug", 0),
                                "engine": ins["engine"],
                                "ins": [],
                                "name": f"{ins['name']}-wsplit{i}",
                                "opcode": "NoOp",
                                "outs": [],
                                "sync_info": {
                                    "on_update": [],
                                    "on_wait": excess[i : i + max_waits],
                                },
                            }
                        )
                    si["on_wait"] = keep
                new.append(ins)
            bb["instructions"] = new
    if not changed:
        return bir_json
    return json.dumps(d).encode()


def _patched_compile_bir_kernel(bir_json, tmpdir, neff_name="file.neff"):
    return compile_bir_kernel(_split_excess_waits(bir_json), tmpdir, neff_name)


bass2jax.compile_bir_kernel = _patched_compile_bir_kernel

BF16 = ml_dtypes.bfloat16
F32 = mybir.dt.float32
BF = mybir.dt.bfloat16

DIM = 2048
T = 2048
HD = 64
N_CORES = 8
AF = mybir.ActivationFunctionType
GROUPS = [[0, 1, 2, 3], [4, 5, 6, 7]]


class _TileContext(tile.TileContext):
    """TileContext whose final drain carries one sem wait per instruction.

    The walrus build in this image rejects a Drain carrying several sync
    waits ("Too many sync wait commands"), so emit individual single-wait
    NOPs on the sync engine first, then an unadorned drain + barriers.
    """

    def _drain_and_barrier(self, tick_clock, wait_clock):
        gc = tick_clock.global_clock
        vals = eval(repr(gc).replace("VectorClock(", "").rstrip(")"))
        for i, v in enumerate(vals):
            if v:
                single = [0] * len(vals)
                single[i] = v
                nop = self.nc.sync.nop(nofuse=True)
                wait_clock.add_sem_waits(
                    nop.ins, ScopedClock({None: VectorClock(single)})
                )
        self.nc.sync.drain()
        self.nc.all_engine_barrier()
        popped = self.nc._tile_sem_poison_stack.pop()
        assert popped is self._sem_poison
        self.nc.clear_and_free_semaphores(list(self.sems.allocated().values()))
        self.nc.all_engine_barrier()


def _build_nc():
    nc = bass.Bass("TRN2")

    xt = nc.declare_dram_parameter("xt", [DIM, T], BF, isOutput=False)
    wq = nc.declare_dram_parameter("wq", [DIM, 512], BF, isOutput=False)
    wkv = nc.declare_dram_parameter("wkv", [DIM, 128], BF, isOutput=False)
    wo = nc.declare_dram_parameter("wo", [512, DIM], BF, isOutput=False)
    cos2 = nc.declare_dram_parameter("cos2", [128, T], BF, isOutput=False)
    sin2 = nc.declare_dram_parameter("sin2", [128, T], BF, isOutput=False)
    r2t = nc.declare_dram_parameter("r2t", [128, 128], BF, isOutput=False)
    ident2 = nc.declare_dram_parameter("ident2", [128, 64], BF, isOutput=False)
    masks = nc.declare_dram_parameter("masks", [128, 128], BF, isOutput=False)
    outb = nc.declare_dram_parameter("outb", [4, 512, 512], BF, isOutput=True)

    with _TileContext(nc) as tc:
        with (
            tc.tile_pool(name="consts", bufs=1) as consts,
            tc.tile_pool(name="big", bufs=1) as big,
            tc.tile_pool(name="wts", bufs=1) as wts,
            tc.tile_pool(name="acts", bufs=1) as acts,
            tc.tile_pool(name="work", bufs=4) as work,
            tc.tile_pool(name="exps", bufs=6) as exps,
            tc.tile_pool(name="outp", bufs=3) as outp,
            tc.tile_pool(name="psum", bufs=2, space="PSUM") as psum,
            tc.tile_pool(name="dram", bufs=1, space="DRAM") as dram,
        ):
            # ---- PE warm-up feeds off an on-chip memset tile so it can
            # start before any DMA lands; ~10us of back-to-back matmuls
            # during the DMA intro lifts the HAM clock gate to 2.4 GHz
            warm_sb = consts.tile([128, 128], BF)
            nc.vector.memset(warm_sb[:], 1.0)
            pwarm = psum.tile([128, 512], F32, tag="mm", name="pwarm")
            for wi in range(100):
                nc.tensor.matmul(
                    pwarm[:, 0:128], lhsT=warm_sb[:], rhs=warm_sb[:],
                    start=True, stop=True,
                )
            # ---- activations / weights in ----
            # the DMA engine drains both HWDGE queues serially, so order
            # transfers by first-use: xt tt0 + wkv (kv proj), then wq + rope
            # tables (q proj), then the rest.  r2t only feeds the rope tail
            # (the warm-up runs off a memset tile), so it loads after wkv.
            wkv_sb = wts.tile([128, 16, 128], BF)
            nc.sync.dma_start(
                wkv_sb[:], wkv[:].rearrange("(fc p) m -> p fc m", p=128)
            )
            r2t_sb = consts.tile([128, 128], BF)
            nc.sync.dma_start(r2t_sb[:], r2t[:])
            masks_sb = consts.tile([128, 128], BF)
            nc.sync.dma_start(masks_sb[:], masks[:])
            ident2_sb = consts.tile([128, 64], BF)
            nc.sync.dma_start(ident2_sb[:], ident2[:])
            cos2_sb = consts.tile([128, T], BF)
            sin2_sb = consts.tile([128, T], BF)
            xt_sb = big.tile([128, 16, T], BF, tag="big")

            # scalar HWDGE queue: xt tt0 first (kv+q proj prologue), then wq
            nc.scalar.dma_start(
                xt_sb[:, :, 0:512],
                xt[:, 0:512].rearrange("(fc p) c -> p fc c", p=128),
            )
            wq_sb = wts.tile([128, 16, 4, 128], BF)
            nc.scalar.dma_start(
                wq_sb[:, 0:8, :, :],
                wq[0:1024, :].rearrange(
                    "(fc p) (qc m) -> p fc qc m", p=128, m=128
                ),
            )
            nc.scalar.dma_start(cos2_sb[:], cos2[:])
            nc.scalar.dma_start(sin2_sb[:], sin2[:])
            nc.scalar.dma_start(
                wq_sb[:, 8:16, :, :],
                wq[1024:2048, :].rearrange(
                    "(fc p) (qc m) -> p fc qc m", p=128, m=128
                ),
            )
            for fc in range(16):
                queue = nc.sync.dma_start if fc % 2 == 0 else nc.scalar.dma_start
                queue(
                    xt_sb[:, fc, 512:T],
                    xt[fc * 128 : fc * 128 + 128, 512:T],
                )
            wo_sb = wts.tile([128, 4, 16, 128], BF)
            for fcl in range(4):
                nc.scalar.dma_start(
                    wo_sb[:, fcl, :, :],
                    wo[fcl * 128 : fcl * 128 + 128, :].rearrange(
                        "p (oc m) -> p oc m", m=128
                    ),
                )

            # ---- on-chip activations ----
            kvrope_sb = acts.tile([128, T], BF)
            kdup_sb = acts.tile([128, T], BF)
            v1_sb = acts.tile([128, 16, 65], BF)
            nc.vector.memset(v1_sb[:, :, 64:65], 1.0)
            ones_sb = consts.tile([65, 64], BF)
            nc.vector.memset(ones_sb[:], 1.0)
            qrope_sb = acts.tile([128, 4, T], BF)
            ao_sb = acts.tile([128, 4, T], BF)

            scale = 1.0 / np.sqrt(HD)

            def kv_unit(tt):
                # kv projection + rope for token slice tt
                # (k rows 0..63, v rows 64..127)
                ts = slice(tt * 512, tt * 512 + 512)
                ps = psum.tile([128, 512], F32, tag="mm")
                for fc in range(16):
                    nc.tensor.matmul(
                        ps[:],
                        lhsT=wkv_sb[:, fc, :],
                        rhs=xt_sb[:, fc, ts],
                        start=(fc == 0),
                        stop=(fc == 15),
                    )
                kv_sb = work.tile([128, 512], BF, tag="evac")
                nc.vector.tensor_copy(kv_sb[:], ps[:])
                psu = psum.tile([128, 512], F32, tag="mm", name="psu")
                nc.tensor.matmul(
                    psu[0:64, :], lhsT=r2t_sb[0:64, 0:64], rhs=kv_sb[0:64, :],
                    start=True, stop=True,
                )
                t1 = work.tile([128, 512], BF, tag="t1")
                nc.vector.tensor_mul(t1[0:64, :], kv_sb[0:64, :], cos2_sb[0:64, ts])
                t2 = work.tile([128, 512], BF, tag="t2")
                nc.vector.tensor_mul(t2[0:64, :], psu[0:64, :], sin2_sb[0:64, ts])
                nc.vector.tensor_add(kvrope_sb[0:64, ts], t1[0:64, :], t2[0:64, :])
                nc.vector.tensor_copy(kvrope_sb[64:128, ts], kv_sb[64:128, :])
                # duplicate roped k into both partition halves
                nc.sync.dma_start(kdup_sb[0:64, ts], kvrope_sb[0:64, ts])
                nc.sync.dma_start(kdup_sb[64:128, ts], kvrope_sb[0:64, ts])
                # v' chunks [128 tok, 65]: col 64 = 1.0 (softmax denom trick)
                for kt in range(4 * tt, 4 * tt + 4):
                    pst = psum.tile([128, 64], BF, tag="pav", bufs=2)
                    nc.tensor.transpose(
                        pst[:],
                        kvrope_sb[64:128, kt * 128 : kt * 128 + 128],
                        ident2_sb[64:128, :],
                    )
                    nc.scalar.copy(v1_sb[:, kt, 0:64], pst[:])

            def q_unit_chunks(tt, ph):
                # q projection + rope for head pair ph, token slice tt,
                # split into filler chunks so exp/attention can overlap
                ts = slice(tt * 512, tt * 512 + 512)
                state = {}

                def acc(c0):
                    def fn():
                        if c0 == 0:
                            state["ps"] = psum.tile(
                                [128, 512], F32, tag="mm", name="psq"
                            )
                        ps = state["ps"]
                        for fc in range(c0, c0 + 4):
                            nc.tensor.matmul(
                                ps[:],
                                lhsT=wq_sb[:, fc, ph, :],
                                rhs=xt_sb[:, fc, ts],
                                start=(fc == 0),
                                stop=(fc == 15),
                            )
                    return fn

                def tail():
                    ps = state["ps"]
                    q_sb = work.tile([128, 512], BF, tag="evac")
                    nc.vector.tensor_copy(q_sb[:], ps[:])
                    psu = psum.tile([128, 512], F32, tag="mm", name="psu2")
                    nc.tensor.matmul(
                        psu[:], lhsT=r2t_sb[:], rhs=q_sb[:],
                        start=True, stop=True,
                    )
                    t1 = work.tile([128, 512], BF, tag="t1")
                    nc.vector.tensor_mul(t1[:], q_sb[:], cos2_sb[:, ts])
                    t2 = work.tile([128, 512], BF, tag="t2")
                    nc.vector.tensor_mul(t2[:], psu[:], sin2_sb[:, ts])
                    nc.vector.tensor_add(qrope_sb[:, ph, ts], t1[:], t2[:])

                return [acc(0), acc(4), acc(8), acc(12), tail]

            def emit_norm(u, last=False):
                # normalize: per-query reciprocal of the denominator row,
                # broadcast across partitions with a K=1 ones-matmul, then
                # one mul per head half straight out of PSUM.  par0 lands
                # in-lane in ao_sb; par1 needs a partition-shifting
                # SBUF->SBUF DMA.
                uph, uqb, upav = u
                uQ0 = uqb * 512
                avu = work.tile([65, 1024], BF, tag="avu")
                if last:
                    # the final unit's norm gates the wo(3) tail: skip the
                    # staging copy (early PSUM release is moot at the end)
                    # and run reciprocal straight off the accumulators; the
                    # multiplies then read pav (one PSUM operand) x bc copied
                    # through SBUF by the idle ACT engine.
                    with nc.allow_low_precision(
                        reason="bf16 softmax denominators are within tolerance"
                    ):
                        nc.vector.reciprocal(avu[64:65, 0:512], upav[0][64:65, :])
                        nc.vector.reciprocal(avu[64:65, 512:1024], upav[1][64:65, :])
                else:
                    nc.vector.tensor_copy(avu[:, 0:512], upav[0][:])
                    nc.vector.tensor_copy(avu[:, 512:1024], upav[1][:])
                    with nc.allow_low_precision(
                        reason="bf16 softmax denominators are within tolerance"
                    ):
                        nc.vector.reciprocal(avu[64:65, :], avu[64:65, :])
                bc = psum.tile([64, 1024], F32, tag="pss", bufs=2, name="bc")
                nc.tensor.matmul(
                    bc[:, 0:512], lhsT=ones_sb[64:65, :],
                    rhs=avu[64:65, 0:512], start=True, stop=True,
                )
                nc.tensor.matmul(
                    bc[:, 512:1024], lhsT=ones_sb[64:65, :],
                    rhs=avu[64:65, 512:1024], start=True, stop=True,
                )
                if last:
                    b_sb = work.tile([64, 1024], BF, tag="bcast")
                    nc.scalar.copy(b_sb[:], bc[:])
                    nc.vector.tensor_mul(
                        ao_sb[0:64, uph, uQ0 : uQ0 + 512],
                        upav[0][0:64, :], b_sb[:, 0:512],
                    )
                    av_sb = work.tile([64, 512], BF, tag="av", name="av1")
                    nc.vector.tensor_mul(
                        av_sb[:], upav[1][0:64, :], b_sb[:, 512:1024]
                    )
                else:
                    nc.vector.tensor_mul(
                        ao_sb[0:64, uph, uQ0 : uQ0 + 512],
                        bc[:, 0:512], avu[0:64, 0:512],
                    )
                    av_sb = work.tile([64, 512], BF, tag="av", name="av1")
                    nc.vector.tensor_mul(
                        av_sb[:], bc[:, 512:1024], avu[0:64, 512:1024]
                    )
                nc.sync.dma_start(
                    ao_sb[64:128, uph, uQ0 : uQ0 + 512], av_sb[:]
                )

            def attn_unit(qb, ph, pending):
                # attention for heads (2*ph, 2*ph+1), query block qb
                Q0 = qb * 512
                nkt = 4 * qb + 4
                pav = [
                    psum.tile([65, 512], F32, tag="pav", name=f"pav{i}", bufs=2)
                    for i in range(2)
                ]
                for pr in range(nkt // 2):
                    kt0, kt1 = 2 * pr, 2 * pr + 1
                    # causal-active widths (tiles above the diagonal shrink)
                    j0, j1 = kt0 - 4 * qb, kt1 - 4 * qb
                    w0 = 512 if j0 < 0 else 512 - 128 * j0
                    w1 = 512 if j1 < 0 else 512 - 128 * j1
                    diag = j0 >= 0
                    # scores for both head halves interleaved so adjacent
                    # matmuls target different PE row groups (concurrent)
                    pss = [
                        psum.tile([128, 1024], F32, tag="pss", bufs=2,
                                  name=f"pss{i}")
                        for i in range(2)
                    ]
                    for kt, w, off in ((kt0, w0, 0), (kt1, w1, w0)):
                        for par in range(2):
                            lo, hi = (0, 64) if par == 0 else (64, 128)
                            nc.tensor.matmul(
                                pss[par][:, off : off + w],
                                lhsT=kdup_sb[lo:hi, kt * 128 : kt * 128 + 128],
                                rhs=qrope_sb[lo:hi, ph, Q0 + 512 - w : Q0 + 512],
                                start=True,
                                stop=True,
                            )
                    e_pair = []
                    for par in range(2):
                        e_sb = exps.tile([128, 1024], BF, tag="e", name=f"e{par}")
                        nc.scalar.activation(
                            e_sb[:, 0 : w0 + w1], pss[par][:, 0 : w0 + w1],
                            AF.Exp, scale=scale,
                        )
                        if diag:
                            # only the first 128 query columns of each slab
                            # overlap the key tile (the tril block); the rest
                            # are fully causal-visible
                            nc.vector.tensor_mul(
                                e_sb[:, 0:128], e_sb[:, 0:128],
                                masks_sb[:, 0:128],
                            )
                            nc.vector.tensor_mul(
                                e_sb[:, w0 : w0 + 128], e_sb[:, w0 : w0 + 128],
                                masks_sb[:, 0:128],
                            )
                        e_pair.append(e_sb)
                    for kt, w, off in ((kt0, w0, 0), (kt1, w1, w0)):
                        for par in range(2):
                            nc.tensor.matmul(
                                pav[par][:, 512 - w : 512],
                                lhsT=v1_sb[:, kt, :],
                                rhs=e_pair[par][:, off : off + w],
                                start=(kt == 0),
                                stop=(kt == nkt - 1),
                            )
                    if pr == min(1, nkt // 2 - 1) and pending is not None:
                        emit_norm(pending)
                        pending = None
                    # qb3 units have 8 prs but only 4 fillers: spread them
                    if qb < 3 or pr % 2 == 1:
                        drain_filler(3 if pr == nkt // 2 - 1 else 1)
                if pending is not None:
                    emit_norm(pending)
                return (ph, qb, pav)

            # wo partial: partial[oc*128.., Q0:Q0+512] =
            #   sum_ph wo_sb[:, ph, oc, :]^T @ ao_sb[:, ph, Q0:Q0+512]
            partial = dram.tile([4, DIM, 512], BF, name="partial")

            wo_state = {}

            def wo_chunk(qb, oc, tail=False):
                # 4-oc groups share one SBUF staging tile and one store DMA.
                # Tail chunks (after the last attention unit) alternate onto
                # the freed pav ring so four ocs can prefetch their ph0-2
                # accumulation while the final norm chain resolves ph3.
                def fn():
                    Q0 = qb * 512
                    tag = "pav" if (tail and oc % 2 == 1) else "mm"
                    ps = psum.tile([128, 512], F32, tag=tag, name="pswo")
                    for ph in range(4):
                        nc.tensor.matmul(
                            ps[:],
                            lhsT=wo_sb[:, ph, oc, :],
                            rhs=ao_sb[:, ph, Q0 : Q0 + 512],
                            start=(ph == 0),
                            stop=(ph == 3),
                        )
                    if oc % 4 == 0:
                        wo_state["o_sb"] = outp.tile([128, 4, 512], BF, tag="o", name="o_sb")
                    o_sb = wo_state["o_sb"]
                    nc.vector.tensor_copy(o_sb[:, oc % 4, :], ps[:])
                    if oc % 4 == 3:
                        oc0 = oc - 3
                        nc.sync.dma_start(
                            partial[qb].rearrange(
                                "(oc p) c -> p oc c", p=128
                            )[:, oc0 : oc0 + 4, :],
                            o_sb[:],
                        )
                return fn

            filler_q = []  # entries: (tag, fn); tag = ("q", tt, ph) | ("wo", qb)

            def drain_filler(n):
                for _ in range(n):
                    if filler_q:
                        filler_q.pop(0)[1]()

            def drain_until(pred):
                # pop (FIFO) until no queued filler matches pred
                while any(pred(t) for t, _ in filler_q):
                    filler_q.pop(0)[1]()

            rsout = dram.tile([4, 512, 512], BF, name="rsout")

            def emit_rs(qb):
                nc.gpsimd.collective_compute(
                    "ReduceScatter",
                    mybir.AluOpType.add,
                    ins=[partial[qb]],
                    outs=[rsout[qb]],
                    replica_groups=GROUPS,
                )
                # the final copy rides the idle SP queue (nothing is queued
                # behind it); earlier ones stay on Pool so SP's partial
                # stores are never blocked behind a waiting collective
                if qb == 3:
                    nc.sync.dma_start(outb[qb], rsout[qb])
                else:
                    nc.gpsimd.dma_start(outb[qb], rsout[qb])

            # ---- emission schedule ----
            kv_unit(0)
            for c in q_unit_chunks(0, 0):
                c()
            for ph in range(1, 4):
                for c in q_unit_chunks(0, ph):
                    c()
            for tt in range(1, 4):
                kv_unit(tt)

            pending = None
            for qb in range(4):
                for ph in range(4):
                    if qb >= 1:
                        filler_q.extend(
                            (("wo", qb - 1), wo_chunk(qb - 1, oc))
                            for oc in range(4 * ph, 4 * ph + 4)
                        )
                    if qb < 3:
                        filler_q.extend(
                            (("q", qb + 1, ph), c)
                            for c in q_unit_chunks(qb + 1, ph)
                        )
                    # q-proj chunks for THIS unit must be emitted before the
                    # scores that read them
                    drain_until(lambda t: t[0] == "q" and t[1:] == (qb, ph))
                    pending = attn_unit(qb, ph, pending)
                if qb >= 1:
                    # wo stores for qb-1 must be emitted before its RS
                    drain_until(lambda t: t == ("wo", qb - 1))
                    emit_rs(qb - 1)
            emit_norm(pending, last=True)
            for oc in range(16):
                wo_chunk(3, oc, tail=True)()
            emit_rs(3)

    return nc


def _host_tables():
    inv_freq = 1.0 / (10000.0 ** (np.arange(0, HD, 2, dtype=np.float32) / HD))
    t = np.arange(T, dtype=np.float32)
    freqs = np.einsum("i,j->ij", t, inv_freq)
    emb = np.concatenate([freqs, freqs], axis=-1)  # [T, 64]
    cosT = np.cos(emb).T.astype(np.float32)  # [64, T]
    sinT = np.sin(emb).T.astype(np.float32)

    cos2 = np.ascontiguousarray(np.vstack([cosT, cosT])).astype(BF16)
    sin2 = np.ascontiguousarray(np.vstack([sinT, sinT])).astype(BF16)

    R = np.zeros((HD, HD), dtype=np.float32)
    for d in range(32):
        R[d, d + 32] = -1.0
        R[d + 32, d] = 1.0
    r2 = np.block([[R, np.zeros_like(R)], [np.zeros_like(R), R]])
    r2t = np.ascontiguousarray(r2.T).astype(BF16)  # lhsT: matmul computes R2 @ rhs

    ident2 = np.vstack([np.eye(HD), np.eye(HD)]).astype(BF16)  # [128, 64]

    r_idx = np.arange(128)[:, None]
    c_idx = np.arange(128)[None, :]
    masks = (c_idx >= r_idx).astype(np.float32).astype(BF16)  # [128, 128]

    return dict(
        cos2=cos2, sin2=sin2, r2t=r2t, ident2=ident2, masks=masks,
    )


_STATE = {}


def _get_nc():
    if "nc" not in _STATE:
        _STATE["nc"] = _build_nc()
        _STATE["tables"] = _host_tables()
    return _STATE["nc"], _STATE["tables"]


def _make_in_maps(x, wq, wk, wv, wo, tables):
    x = np.asarray(x, dtype=np.float32)
    wq_b = np.asarray(wq, dtype=np.float32).astype(BF16)
    wo_b = np.asarray(wo, dtype=np.float32).astype(BF16)
    wk_b = np.asarray(wk, dtype=np.float32).astype(BF16)
    wv_b = np.asarray(wv, dtype=np.float32).astype(BF16)

    in_maps = []
    xt_b = [np.ascontiguousarray(x[b].T).astype(BF16) for b in range(2)]
    for core in range(N_CORES):
        b, g = core // 4, core % 4
        m = dict(tables)
        m["xt"] = xt_b[b]
        m["wq"] = np.ascontiguousarray(wq_b[:, 512 * g : 512 * g + 512])
        m["wkv"] = np.ascontiguousarray(
            np.concatenate(
                [wk_b[:, 64 * g : 64 * g + 64], wv_b[:, 64 * g : 64 * g + 64]],
                axis=1,
            )
        )
        m["wo"] = np.ascontiguousarray(wo_b[512 * g : 512 * g + 512, :])
        in_maps.append(m)
    return in_maps


def _assemble(results):
    out = np.empty((2, T, DIM), dtype=np.float32)
    for core in range(N_CORES):
        b, g = core // 4, core % 4
        ob = results[core]["outb"]  # [4 qb, 512 outfeat, 512 tok]
        out[b][:, 512 * g : 512 * g + 512] = (
            np.concatenate(list(ob), axis=1).T.astype(np.float32)
        )
    return out


def kernel(x, wq, wk, wv, wo):
    nc, tables = _get_nc()
    in_maps = _make_in_maps(x, wq, wk, wv, wo, tables)
    res = run_bass_kernel_spmd(
        nc, in_maps, core_ids=list(range(N_CORES)), trace=False
    )
    return _assemble(res.results)


# revision 95
# speedup vs baseline: 1.0020x; 1.0020x over previous
"""Distributed GQA attention block (dense_transformer) for 8 TRN2 NeuronCores.

Reference computation (all fp32):
    q = (x @ wq)  -> RoPE;  k = (x @ wk) -> RoPE;  v = x @ wv
    causal softmax(q k^T / sqrt(64)) @ v  (GQA: 32 q heads, 4 kv heads)
    out = attn_out @ wo

Sharding: core (b, g) for b in {0,1}, g in {0..3} handles batch b, q-heads
8g..8g+7, kv-head g (data-parallel over batch x tensor-parallel over GQA
groups).  Each core computes attn_out for its heads ([512, 2048],
feature-major) and a full-width partial output
partial = wo[512g:512g+512, :]^T @ attn_out  ([2048, T]); a ReduceScatter
(add) per 512-token block sums the partials within the 4-core batch group
and leaves each core with its disjoint 512-row slice of the final output.
Attention runs token-block-outer (qb) so each block's partial + RS is
pipelined behind the later blocks' compute; no AllGather, attn_out never
leaves SBUF.

All activations/weights are kept feature-major (transposed) on chip so every
matmul contracts over the partition dim with no on-chip transposes except a
single small one for v.  Matmul compute in bf16 (fp32 PSUM accumulate).
"""

import json

import numpy as np
import ml_dtypes

import concourse.bass as bass
import concourse.bass2jax as bass2jax
import concourse.mybir as mybir
import concourse.tile as tile
from concourse.tile import VectorClock, ScopedClock
from concourse.bass_utils import compile_bir_kernel, run_bass_kernel_spmd

_MAX_WAITS = 1  # this walrus build rejects instructions with more sem waits


def _split_excess_waits(bir_json, max_waits=_MAX_WAITS):
    """Hoist excess per-instruction sem waits onto injected same-engine NoOps.

    The TRN2 ISA encoding in this neuronxcc build allows at most `max_waits`
    sync-wait commands per instruction; Tile's sem assigner can emit more.
    A NoOp inserted immediately before the instruction on the same engine is
    semantically identical (the engine blocks at the same program point).
    """
    d = json.loads(bir_json)
    changed = False
    for fn in d.get("functions", []):
        for bb in fn.get("blocks", []):
            insts = bb.get("instructions", [])
            new = []
            for ins in insts:
                si = ins.get("sync_info")
                waits = (si or {}).get("on_wait") or []
                if len(waits) > max_waits:
                    changed = True
                    excess, keep = waits[:-max_waits], waits[-max_waits:]
                    for i in range(0, len(excess), max_waits):
                        new.append(
                            {
                                "debug": ins.get("debug", 0),
                                "engine": ins["engine"],
                                "ins": [],
                                "name": f"{ins['name']}-wsplit{i}",
                                "opcode": "NoOp",
                                "outs": [],
                                "sync_info": {
                                    "on_update": [],
                                    "on_wait": excess[i : i + max_waits],
                                },
                            }
                        )
                    si["on_wait"] = keep
                new.append(ins)
            bb["instructions"] = new
    if not changed:
        return bir_json
    return json.dumps(d).encode()


def _patched_compile_bir_kernel(bir_json, tmpdir, neff_name="file.neff"):
    return compile_bir_kernel(_split_excess_waits(bir_json), tmpdir, neff_name)


bass2jax.compile_bir_kernel = _patched_compile_bir_kernel

BF16 = ml_dtypes.bfloat16
F32 = mybir.dt.float32
BF = mybir.dt.bfloat16

DIM = 2048
T = 2048
HD = 64
N_CORES = 8
AF = mybir.ActivationFunctionType
GROUPS = [[0, 1, 2, 3], [4, 5, 6, 7]]


class _TileContext(tile.TileContext):
    """TileContext whose final drain carries one sem wait per instruction.

    The walrus build in this image rejects a Drain carrying several sync
    waits ("Too many sync wait commands"), so emit individual single-wait
    NOPs on the sync engine first, then an unadorned drain + barriers.
    """

    def _drain_and_barrier(self, tick_clock, wait_clock):
        gc = tick_clock.global_clock
        vals = eval(repr(gc).replace("VectorClock(", "").rstrip(")"))
        for i, v in enumerate(vals):
            if v:
                single = [0] * len(vals)
                single[i] = v
                nop = self.nc.sync.nop(nofuse=True)
                wait_clock.add_sem_waits(
                    nop.ins, ScopedClock({None: VectorClock(single)})
                )
        self.nc.sync.drain()
        self.nc.all_engine_barrier()
        popped = self.nc._tile_sem_poison_stack.pop()
        assert popped is self._sem_poison
        self.nc.clear_and_free_semaphores(list(self.sems.allocated().values()))
        self.nc.all_engine_barrier()


def _build_nc():
    nc = bass.Bass("TRN2")

    xt = nc.declare_dram_parameter("xt", [DIM, T], BF, isOutput=False)
    wq = nc.declare_dram_parameter("wq", [DIM, 512], BF, isOutput=False)
    wkv = nc.declare_dram_parameter("wkv", [DIM, 128], BF, isOutput=False)
    wo = nc.declare_dram_parameter("wo", [512, DIM], BF, isOutput=False)
    cos2 = nc.declare_dram_parameter("cos2", [64, T], BF, isOutput=False)
    sin2 = nc.declare_dram_parameter("sin2", [64, T], BF, isOutput=False)
    r2t = nc.declare_dram_parameter("r2t", [128, 128], BF, isOutput=False)
    ident2 = nc.declare_dram_parameter("ident2", [128, 64], BF, isOutput=False)
    masks = nc.declare_dram_parameter("masks", [128, 128], BF, isOutput=False)
    outb = nc.declare_dram_parameter("outb", [4, 512, 512], BF, isOutput=True)

    with _TileContext(nc) as tc:
        with (
            tc.tile_pool(name="consts", bufs=1) as consts,
            tc.tile_pool(name="big", bufs=1) as big,
            tc.tile_pool(name="wts", bufs=1) as wts,
            tc.tile_pool(name="acts", bufs=1) as acts,
            tc.tile_pool(name="work", bufs=4) as work,
            tc.tile_pool(name="exps", bufs=6) as exps,
            tc.tile_pool(name="outp", bufs=3) as outp,
            tc.tile_pool(name="psum", bufs=2, space="PSUM") as psum,
            tc.tile_pool(name="dram", bufs=1, space="DRAM") as dram,
        ):
            # ---- PE warm-up feeds off an on-chip memset tile so it can
            # start before any DMA lands; ~10us of back-to-back matmuls
            # during the DMA intro lifts the HAM clock gate to 2.4 GHz
            warm_sb = consts.tile([128, 128], BF)
            nc.vector.memset(warm_sb[:], 1.0)
            pwarm = psum.tile([128, 512], F32, tag="mm", name="pwarm")
            for wi in range(100):
                nc.tensor.matmul(
                    pwarm[:, 0:128], lhsT=warm_sb[:], rhs=warm_sb[:],
                    start=True, stop=True,
                )
            # ---- activations / weights in ----
            # the DMA engine drains both HWDGE queues serially, so order
            # transfers by first-use: xt tt0 + wkv (kv proj), then wq + rope
            # tables (q proj), then the rest.  r2t only feeds the rope tail
            # (the warm-up runs off a memset tile), so it loads after wkv.
            wkv_sb = wts.tile([128, 16, 128], BF)
            nc.sync.dma_start(
                wkv_sb[:], wkv[:].rearrange("(fc p) m -> p fc m", p=128)
            )
            r2t_sb = consts.tile([128, 128], BF)
            nc.sync.dma_start(r2t_sb[:], r2t[:])
            masks_sb = consts.tile([128, 128], BF)
            nc.sync.dma_start(masks_sb[:], masks[:])
            ident2_sb = consts.tile([128, 64], BF)
            nc.sync.dma_start(ident2_sb[:], ident2[:])
            cos2_sb = consts.tile([128, T], BF)
            sin2_sb = consts.tile([128, T], BF)
            xt_sb = big.tile([128, 16, T], BF, tag="big")

            # scalar HWDGE queue: xt tt0 first (kv+q proj prologue), then wq
            nc.scalar.dma_start(
                xt_sb[:, :, 0:512],
                xt[:, 0:512].rearrange("(fc p) c -> p fc c", p=128),
            )
            wq_sb = wts.tile([128, 16, 4, 128], BF)
            nc.scalar.dma_start(
                wq_sb[:, 0:8, :, :],
                wq[0:1024, :].rearrange(
                    "(fc p) (qc m) -> p fc qc m", p=128, m=128
                ),
            )
            # tables are identical in both partition halves: load 64 rows
            # and duplicate on-chip (halves the serial-DMA-engine cost)
            nc.scalar.dma_start(cos2_sb[0:64, :], cos2[:])
            nc.scalar.dma_start(sin2_sb[0:64, :], sin2[:])
            nc.scalar.dma_start(cos2_sb[64:128, :], cos2_sb[0:64, :])
            nc.scalar.dma_start(sin2_sb[64:128, :], sin2_sb[0:64, :])
            nc.scalar.dma_start(
                wq_sb[:, 8:16, :, :],
                wq[1024:2048, :].rearrange(
                    "(fc p) (qc m) -> p fc qc m", p=128, m=128
                ),
            )
            for fc in range(16):
                queue = nc.sync.dma_start if fc % 2 == 0 else nc.scalar.dma_start
                queue(
                    xt_sb[:, fc, 512:T],
                    xt[fc * 128 : fc * 128 + 128, 512:T],
                )
            wo_sb = wts.tile([128, 4, 16, 128], BF)
            for fcl in range(4):
                nc.scalar.dma_start(
                    wo_sb[:, fcl, :, :],
                    wo[fcl * 128 : fcl * 128 + 128, :].rearrange(
                        "p (oc m) -> p oc m", m=128
                    ),
                )

            # ---- on-chip activations ----
            kvrope_sb = acts.tile([128, T], BF)
            kdup_sb = acts.tile([128, T], BF)
            v1_sb = acts.tile([128, 16, 65], BF)
            nc.vector.memset(v1_sb[:, :, 64:65], 1.0)
            ones_sb = consts.tile([65, 64], BF)
            nc.vector.memset(ones_sb[:], 1.0)
            qrope_sb = acts.tile([128, 4, T], BF)
            ao_sb = acts.tile([128, 4, T], BF)

            scale = 1.0 / np.sqrt(HD)

            def kv_unit(tt):
                # kv projection + rope for token slice tt
                # (k rows 0..63, v rows 64..127)
                ts = slice(tt * 512, tt * 512 + 512)
                ps = psum.tile([128, 512], F32, tag="mm")
                for fc in range(16):
                    nc.tensor.matmul(
                        ps[:],
                        lhsT=wkv_sb[:, fc, :],
                        rhs=xt_sb[:, fc, ts],
                        start=(fc == 0),
                        stop=(fc == 15),
                    )
                kv_sb = work.tile([128, 512], BF, tag="evac")
                nc.vector.tensor_copy(kv_sb[:], ps[:])
                psu = psum.tile([128, 512], F32, tag="mm", name="psu")
                nc.tensor.matmul(
                    psu[0:64, :], lhsT=r2t_sb[0:64, 0:64], rhs=kv_sb[0:64, :],
                    start=True, stop=True,
                )
                t1 = work.tile([128, 512], BF, tag="t1")
                nc.vector.tensor_mul(t1[0:64, :], kv_sb[0:64, :], cos2_sb[0:64, ts])
                t2 = work.tile([128, 512], BF, tag="t2")
                nc.vector.tensor_mul(t2[0:64, :], psu[0:64, :], sin2_sb[0:64, ts])
                nc.vector.tensor_add(kvrope_sb[0:64, ts], t1[0:64, :], t2[0:64, :])
                nc.vector.tensor_copy(kvrope_sb[64:128, ts], kv_sb[64:128, :])
                # duplicate roped k into both partition halves
                nc.sync.dma_start(kdup_sb[0:64, ts], kvrope_sb[0:64, ts])
                nc.sync.dma_start(kdup_sb[64:128, ts], kvrope_sb[0:64, ts])
                # v' chunks [128 tok, 65]: col 64 = 1.0 (softmax denom trick)
                for kt in range(4 * tt, 4 * tt + 4):
                    pst = psum.tile([128, 64], BF, tag="pav", bufs=2)
                    nc.tensor.transpose(
                        pst[:],
                        kvrope_sb[64:128, kt * 128 : kt * 128 + 128],
                        ident2_sb[64:128, :],
                    )
                    nc.scalar.copy(v1_sb[:, kt, 0:64], pst[:])

            def q_unit_chunks(tt, ph):
                # q projection + rope for head pair ph, token slice tt,
                # split into filler chunks so exp/attention can overlap
                ts = slice(tt * 512, tt * 512 + 512)
                state = {}

                def acc(c0):
                    def fn():
                        if c0 == 0:
                            state["ps"] = psum.tile(
                                [128, 512], F32, tag="mm", name="psq"
                            )
                        ps = state["ps"]
                        for fc in range(c0, c0 + 4):
                            nc.tensor.matmul(
                                ps[:],
                                lhsT=wq_sb[:, fc, ph, :],
                                rhs=xt_sb[:, fc, ts],
                                start=(fc == 0),
                                stop=(fc == 15),
                            )
                    return fn

                def tail():
                    ps = state["ps"]
                    q_sb = work.tile([128, 512], BF, tag="evac")
                    nc.vector.tensor_copy(q_sb[:], ps[:])
                    psu = psum.tile([128, 512], F32, tag="mm", name="psu2")
                    nc.tensor.matmul(
                        psu[:], lhsT=r2t_sb[:], rhs=q_sb[:],
                        start=True, stop=True,
                    )
                    t1 = work.tile([128, 512], BF, tag="t1")
                    nc.vector.tensor_mul(t1[:], q_sb[:], cos2_sb[:, ts])
                    t2 = work.tile([128, 512], BF, tag="t2")
                    nc.vector.tensor_mul(t2[:], psu[:], sin2_sb[:, ts])
                    nc.vector.tensor_add(qrope_sb[:, ph, ts], t1[:], t2[:])

                return [acc(0), acc(4), acc(8), acc(12), tail]

            def emit_norm(u, last=False):
                # normalize: per-query reciprocal of the denominator row,
                # broadcast across partitions with a K=1 ones-matmul, then
                # one mul per head half straight out of PSUM.  par0 lands
                # in-lane in ao_sb; par1 needs a partition-shifting
                # SBUF->SBUF DMA.
                uph, uqb, upav = u
                uQ0 = uqb * 512
                avu = work.tile([65, 1024], BF, tag="avu")
                if last:
                    # the final unit's norm gates the wo(3) tail: skip the
                    # staging copy (early PSUM release is moot at the end)
                    # and run reciprocal straight off the accumulators; the
                    # multiplies then read pav (one PSUM operand) x bc copied
                    # through SBUF by the idle ACT engine.
                    with nc.allow_low_precision(
                        reason="bf16 softmax denominators are within tolerance"
                    ):
                        nc.vector.reciprocal(avu[64:65, 0:512], upav[0][64:65, :])
                        nc.vector.reciprocal(avu[64:65, 512:1024], upav[1][64:65, :])
                else:
                    nc.vector.tensor_copy(avu[:, 0:512], upav[0][:])
                    nc.vector.tensor_copy(avu[:, 512:1024], upav[1][:])
                    with nc.allow_low_precision(
                        reason="bf16 softmax denominators are within tolerance"
                    ):
                        nc.vector.reciprocal(avu[64:65, :], avu[64:65, :])
                bc = psum.tile([64, 1024], F32, tag="pss", bufs=2, name="bc")
                nc.tensor.matmul(
                    bc[:, 0:512], lhsT=ones_sb[64:65, :],
                    rhs=avu[64:65, 0:512], start=True, stop=True,
                )
                nc.tensor.matmul(
                    bc[:, 512:1024], lhsT=ones_sb[64:65, :],
                    rhs=avu[64:65, 512:1024], start=True, stop=True,
                )
                if last:
                    b_sb = work.tile([64, 1024], BF, tag="bcast")
                    nc.scalar.copy(b_sb[:], bc[:])
                    nc.vector.tensor_mul(
                        ao_sb[0:64, uph, uQ0 : uQ0 + 512],
                        upav[0][0:64, :], b_sb[:, 0:512],
                    )
                    av_sb = work.tile([64, 512], BF, tag="av", name="av1")
                    nc.vector.tensor_mul(
                        av_sb[:], upav[1][0:64, :], b_sb[:, 512:1024]
                    )
                else:
                    nc.vector.tensor_mul(
                        ao_sb[0:64, uph, uQ0 : uQ0 + 512],
                        bc[:, 0:512], avu[0:64, 0:512],
                    )
                    av_sb = work.tile([64, 512], BF, tag="av", name="av1")
                    nc.vector.tensor_mul(
                        av_sb[:], bc[:, 512:1024], avu[0:64, 512:1024]
                    )
                nc.sync.dma_start(
                    ao_sb[64:128, uph, uQ0 : uQ0 + 512], av_sb[:]
                )

            def attn_unit(qb, ph, pending):
                # attention for heads (2*ph, 2*ph+1), query block qb
                Q0 = qb * 512
                nkt = 4 * qb + 4
                pav = [
                    psum.tile([65, 512], F32, tag="pav", name=f"pav{i}", bufs=2)
                    for i in range(2)
                ]
                for pr in range(nkt // 2):
                    kt0, kt1 = 2 * pr, 2 * pr + 1
                    # causal-active widths (tiles above the diagonal shrink)
                    j0, j1 = kt0 - 4 * qb, kt1 - 4 * qb
                    w0 = 512 if j0 < 0 else 512 - 128 * j0
                    w1 = 512 if j1 < 0 else 512 - 128 * j1
                    diag = j0 >= 0
                    # scores for both head halves interleaved so adjacent
                    # matmuls target different PE row groups (concurrent)
                    pss = [
                        psum.tile([128, 1024], F32, tag="pss", bufs=2,
                                  name=f"pss{i}")
                        for i in range(2)
                    ]
                    for kt, w, off in ((kt0, w0, 0), (kt1, w1, w0)):
                        for par in range(2):
                            lo, hi = (0, 64) if par == 0 else (64, 128)
                            nc.tensor.matmul(
                                pss[par][:, off : off + w],
                                lhsT=kdup_sb[lo:hi, kt * 128 : kt * 128 + 128],
                                rhs=qrope_sb[lo:hi, ph, Q0 + 512 - w : Q0 + 512],
                                start=True,
                                stop=True,
                            )
                    e_pair = []
                    for par in range(2):
                        e_sb = exps.tile([128, 1024], BF, tag="e", name=f"e{par}")
                        nc.scalar.activation(
                            e_sb[:, 0 : w0 + w1], pss[par][:, 0 : w0 + w1],
                            AF.Exp, scale=scale,
                        )
                        if diag:
                            # only the first 128 query columns of each slab
                            # overlap the key tile (the tril block); the rest
                            # are fully causal-visible
                            nc.vector.tensor_mul(
                                e_sb[:, 0:128], e_sb[:, 0:128],
                                masks_sb[:, 0:128],
                            )
                            nc.vector.tensor_mul(
                                e_sb[:, w0 : w0 + 128], e_sb[:, w0 : w0 + 128],
                                masks_sb[:, 0:128],
                            )
                        e_pair.append(e_sb)
                    for kt, w, off in ((kt0, w0, 0), (kt1, w1, w0)):
                        for par in range(2):
                            nc.tensor.matmul(
                                pav[par][:, 512 - w : 512],
                                lhsT=v1_sb[:, kt, :],
                                rhs=e_pair[par][:, off : off + w],
                                start=(kt == 0),
                                stop=(kt == nkt - 1),
                            )
                    if pr == min(1, nkt // 2 - 1) and pending is not None:
                        emit_norm(pending)
                        pending = None
                    # qb3 units have 8 prs but only 4 fillers: spread them
                    if qb < 3 or pr % 2 == 1:
                        drain_filler(3 if pr == nkt // 2 - 1 else 1)
                if pending is not None:
                    emit_norm(pending)
                return (ph, qb, pav)

            # wo partial: partial[oc*128.., Q0:Q0+512] =
            #   sum_ph wo_sb[:, ph, oc, :]^T @ ao_sb[:, ph, Q0:Q0+512]
            partial = dram.tile([4, DIM, 512], BF, name="partial")

            wo_state = {}

            def wo_chunk(qb, oc, tail=False):
                # 4-oc groups share one SBUF staging tile and one store DMA.
                # Tail chunks (after the last attention unit) alternate onto
                # the freed pav ring so four ocs can prefetch their ph0-2
                # accumulation while the final norm chain resolves ph3.
                def fn():
                    Q0 = qb * 512
                    tag = "pav" if (tail and oc % 2 == 1) else "mm"
                    ps = psum.tile([128, 512], F32, tag=tag, name="pswo")
                    for ph in range(4):
                        nc.tensor.matmul(
                            ps[:],
                            lhsT=wo_sb[:, ph, oc, :],
                            rhs=ao_sb[:, ph, Q0 : Q0 + 512],
                            start=(ph == 0),
                            stop=(ph == 3),
                        )
                    if oc % 4 == 0:
                        wo_state["o_sb"] = outp.tile([128, 4, 512], BF, tag="o", name="o_sb")
                    o_sb = wo_state["o_sb"]
                    nc.vector.tensor_copy(o_sb[:, oc % 4, :], ps[:])
                    if oc % 4 == 3:
                        oc0 = oc - 3
                        nc.sync.dma_start(
                            partial[qb].rearrange(
                                "(oc p) c -> p oc c", p=128
                            )[:, oc0 : oc0 + 4, :],
                            o_sb[:],
                        )
                return fn

            filler_q = []  # entries: (tag, fn); tag = ("q", tt, ph) | ("wo", qb)

            def drain_filler(n):
                for _ in range(n):
                    if filler_q:
                        filler_q.pop(0)[1]()

            def drain_until(pred):
                # pop (FIFO) until no queued filler matches pred
                while any(pred(t) for t, _ in filler_q):
                    filler_q.pop(0)[1]()

            rsout = dram.tile([4, 512, 512], BF, name="rsout")

            def emit_rs(qb):
                nc.gpsimd.collective_compute(
                    "ReduceScatter",
                    mybir.AluOpType.add,
                    ins=[partial[qb]],
                    outs=[rsout[qb]],
                    replica_groups=GROUPS,
                )
                # the final copy rides the idle SP queue (nothing is queued
                # behind it); earlier ones stay on Pool so SP's partial
                # stores are never blocked behind a waiting collective
                if qb == 3:
                    nc.sync.dma_start(outb[qb], rsout[qb])
                else:
                    nc.gpsimd.dma_start(outb[qb], rsout[qb])

            # ---- emission schedule ----
            kv_unit(0)
            for c in q_unit_chunks(0, 0):
                c()
            for ph in range(1, 4):
                for c in q_unit_chunks(0, ph):
                    c()
            for tt in range(1, 4):
                kv_unit(tt)

            pending = None
            for qb in range(4):
                for ph in range(4):
                    if qb >= 1:
                        filler_q.extend(
                            (("wo", qb - 1), wo_chunk(qb - 1, oc))
                            for oc in range(4 * ph, 4 * ph + 4)
                        )
                    if qb < 3:
                        filler_q.extend(
                            (("q", qb + 1, ph), c)
                            for c in q_unit_chunks(qb + 1, ph)
                        )
                    # q-proj chunks for THIS unit must be emitted before the
                    # scores that read them
                    drain_until(lambda t: t[0] == "q" and t[1:] == (qb, ph))
                    pending = attn_unit(qb, ph, pending)
                if qb >= 1:
                    # wo stores for qb-1 must be emitted before its RS
                    drain_until(lambda t: t == ("wo", qb - 1))
                    emit_rs(qb - 1)
            emit_norm(pending, last=True)
            for oc in range(16):
                wo_chunk(3, oc, tail=True)()
            emit_rs(3)

    return nc


def _host_tables():
    inv_freq = 1.0 / (10000.0 ** (np.arange(0, HD, 2, dtype=np.float32) / HD))
    t = np.arange(T, dtype=np.float32)
    freqs = np.einsum("i,j->ij", t, inv_freq)
    emb = np.concatenate([freqs, freqs], axis=-1)  # [T, 64]
    cosT = np.cos(emb).T.astype(np.float32)  # [64, T]
    sinT = np.sin(emb).T.astype(np.float32)

    cos2 = np.ascontiguousarray(cosT).astype(BF16)
    sin2 = np.ascontiguousarray(sinT).astype(BF16)

    R = np.zeros((HD, HD), dtype=np.float32)
    for d in range(32):
        R[d, d + 32] = -1.0
        R[d + 32, d] = 1.0
    r2 = np.block([[R, np.zeros_like(R)], [np.zeros_like(R), R]])
    r2t = np.ascontiguousarray(r2.T).astype(BF16)  # lhsT: matmul computes R2 @ rhs

    ident2 = np.vstack([np.eye(HD), np.eye(HD)]).astype(BF16)  # [128, 64]

    r_idx = np.arange(128)[:, None]
    c_idx = np.arange(128)[None, :]
    masks = (c_idx >= r_idx).astype(np.float32).astype(BF16)  # [128, 128]

    return dict(
        cos2=cos2, sin2=sin2, r2t=r2t, ident2=ident2, masks=masks,
    )


_STATE = {}


def _get_nc():
    if "nc" not in _STATE:
        _STATE["nc"] = _build_nc()
        _STATE["tables"] = _host_tables()
    return _STATE["nc"], _STATE["tables"]


def _make_in_maps(x, wq, wk, wv, wo, tables):
    x = np.asarray(x, dtype=np.float32)
    wq_b = np.asarray(wq, dtype=np.float32).astype(BF16)
    wo_b = np.asarray(wo, dtype=np.float32).astype(BF16)
    wk_b = np.asarray(wk, dtype=np.float32).astype(BF16)
    wv_b = np.asarray(wv, dtype=np.float32).astype(BF16)

    in_maps = []
    xt_b = [np.ascontiguousarray(x[b].T).astype(BF16) for b in range(2)]
    for core in range(N_CORES):
        b, g = core // 4, core % 4
        m = dict(tables)
        m["xt"] = xt_b[b]
        m["wq"] = np.ascontiguousarray(wq_b[:, 512 * g : 512 * g + 512])
        m["wkv"] = np.ascontiguousarray(
            np.concatenate(
                [wk_b[:, 64 * g : 64 * g + 64], wv_b[:, 64 * g : 64 * g + 64]],
                axis=1,
            )
        )
        m["wo"] = np.ascontiguousarray(wo_b[512 * g : 512 * g + 512, :])
        in_maps.append(m)
    return in_maps


def _assemble(results):
    out = np.empty((2, T, DIM), dtype=np.float32)
    for core in range(N_CORES):
        b, g = core // 4, core % 4
        ob = results[core]["outb"]  # [4 qb, 512 outfeat, 512 tok]
        out[b][:, 512 * g : 512 * g + 512] = (
            np.concatenate(list(ob), axis=1).T.astype(np.float32)
        )
    return out


def kernel(x, wq, wk, wv, wo):
    nc, tables = _get_nc()
    in_maps = _make_in_maps(x, wq, wk, wv, wo, tables)
    res = run_bass_kernel_spmd(
        nc, in_maps, core_ids=list(range(N_CORES)), trace=False
    )
    return _assemble(res.results)


# revision 96
# speedup vs baseline: 1.0025x; 1.0005x over previous
"""Distributed GQA attention block (dense_transformer) for 8 TRN2 NeuronCores.

Reference computation (all fp32):
    q = (x @ wq)  -> RoPE;  k = (x @ wk) -> RoPE;  v = x @ wv
    causal softmax(q k^T / sqrt(64)) @ v  (GQA: 32 q heads, 4 kv heads)
    out = attn_out @ wo

Sharding: core (b, g) for b in {0,1}, g in {0..3} handles batch b, q-heads
8g..8g+7, kv-head g (data-parallel over batch x tensor-parallel over GQA
groups).  Each core computes attn_out for its heads ([512, 2048],
feature-major) and a full-width partial output
partial = wo[512g:512g+512, :]^T @ attn_out  ([2048, T]); a ReduceScatter
(add) per 512-token block sums the partials within the 4-core batch group
and leaves each core with its disjoint 512-row slice of the final output.
Attention runs token-block-outer (qb) so each block's partial + RS is
pipelined behind the later blocks' compute; no AllGather, attn_out never
leaves SBUF.

All activations/weights are kept feature-major (transposed) on chip so every
matmul contracts over the partition dim with no on-chip transposes except a
single small one for v.  Matmul compute in bf16 (fp32 PSUM accumulate).
"""

import json

import numpy as np
import ml_dtypes

import concourse.bass as bass
import concourse.bass2jax as bass2jax
import concourse.mybir as mybir
import concourse.tile as tile
from concourse.tile import VectorClock, ScopedClock
from concourse.bass_utils import compile_bir_kernel, run_bass_kernel_spmd

_MAX_WAITS = 1  # this walrus build rejects instructions with more sem waits


def _split_excess_waits(bir_json, max_waits=_MAX_WAITS):
    """Hoist excess per-instruction sem waits onto injected same-engine NoOps.

    The TRN2 ISA encoding in this neuronxcc build allows at most `max_waits`
    sync-wait commands per instruction; Tile's sem assigner can emit more.
    A NoOp inserted immediately before the instruction on the same engine is
    semantically identical (the engine blocks at the same program point).
    """
    d = json.loads(bir_json)
    changed = False
    for fn in d.get("functions", []):
        for bb in fn.get("blocks", []):
            insts = bb.get("instructions", [])
            new = []
            for ins in insts:
                si = ins.get("sync_info")
                waits = (si or {}).get("on_wait") or []
                if len(waits) > max_waits:
                    changed = True
                    excess, keep = waits[:-max_waits], waits[-max_waits:]
                    for i in range(0, len(excess), max_waits):
                        new.append(
                            {
                                "debug": ins.get("debug", 0),
                                "engine": ins["engine"],
                                "ins": [],
                                "name": f"{ins['name']}-wsplit{i}",
                                "opcode": "NoOp",
                                "outs": [],
                                "sync_info": {
                                    "on_update": [],
                                    "on_wait": excess[i : i + max_waits],
                                },
                            }
                        )
                    si["on_wait"] = keep
                new.append(ins)
            bb["instructions"] = new
    if not changed:
        return bir_json
    return json.dumps(d).encode()


def _patched_compile_bir_kernel(bir_json, tmpdir, neff_name="file.neff"):
    return compile_bir_kernel(_split_excess_waits(bir_json), tmpdir, neff_name)


bass2jax.compile_bir_kernel = _patched_compile_bir_kernel

BF16 = ml_dtypes.bfloat16
F32 = mybir.dt.float32
BF = mybir.dt.bfloat16

DIM = 2048
T = 2048
HD = 64
N_CORES = 8
AF = mybir.ActivationFunctionType
GROUPS = [[0, 1, 2, 3], [4, 5, 6, 7]]


class _TileContext(tile.TileContext):
    """TileContext whose final drain carries one sem wait per instruction.

    The walrus build in this image rejects a Drain carrying several sync
    waits ("Too many sync wait commands"), so emit individual single-wait
    NOPs on the sync engine first, then an unadorned drain + barriers.
    """

    def _drain_and_barrier(self, tick_clock, wait_clock):
        gc = tick_clock.global_clock
        vals = eval(repr(gc).replace("VectorClock(", "").rstrip(")"))
        for i, v in enumerate(vals):
            if v:
                single = [0] * len(vals)
                single[i] = v
                nop = self.nc.sync.nop(nofuse=True)
                wait_clock.add_sem_waits(
                    nop.ins, ScopedClock({None: VectorClock(single)})
                )
        self.nc.sync.drain()
        self.nc.all_engine_barrier()
        popped = self.nc._tile_sem_poison_stack.pop()
        assert popped is self._sem_poison
        self.nc.clear_and_free_semaphores(list(self.sems.allocated().values()))
        self.nc.all_engine_barrier()


def _build_nc():
    nc = bass.Bass("TRN2")

    xt = nc.declare_dram_parameter("xt", [DIM, T], BF, isOutput=False)
    wq = nc.declare_dram_parameter("wq", [DIM, 512], BF, isOutput=False)
    wkv = nc.declare_dram_parameter("wkv", [DIM, 128], BF, isOutput=False)
    wo = nc.declare_dram_parameter("wo", [512, DIM], BF, isOutput=False)
    cos2 = nc.declare_dram_parameter("cos2", [64, T], BF, isOutput=False)
    sin2 = nc.declare_dram_parameter("sin2", [64, T], BF, isOutput=False)
    r2t = nc.declare_dram_parameter("r2t", [128, 128], BF, isOutput=False)
    ident2 = nc.declare_dram_parameter("ident2", [128, 64], BF, isOutput=False)
    masks = nc.declare_dram_parameter("masks", [128, 128], BF, isOutput=False)
    outb = nc.declare_dram_parameter("outb", [4, 512, 512], BF, isOutput=True)

    with _TileContext(nc) as tc:
        with (
            tc.tile_pool(name="consts", bufs=1) as consts,
            tc.tile_pool(name="big", bufs=1) as big,
            tc.tile_pool(name="wts", bufs=1) as wts,
            tc.tile_pool(name="acts", bufs=1) as acts,
            tc.tile_pool(name="work", bufs=4) as work,
            tc.tile_pool(name="exps", bufs=6) as exps,
            tc.tile_pool(name="outp", bufs=3) as outp,
            tc.tile_pool(name="psum", bufs=2, space="PSUM") as psum,
            tc.tile_pool(name="dram", bufs=1, space="DRAM") as dram,
        ):
            # ---- PE warm-up feeds off an on-chip memset tile so it can
            # start before any DMA lands; ~10us of back-to-back matmuls
            # during the DMA intro lifts the HAM clock gate to 2.4 GHz
            warm_sb = consts.tile([128, 128], BF)
            nc.vector.memset(warm_sb[:], 1.0)
            pwarm = psum.tile([128, 512], F32, tag="mm", name="pwarm")
            for wi in range(100):
                nc.tensor.matmul(
                    pwarm[:, 0:128], lhsT=warm_sb[:], rhs=warm_sb[:],
                    start=True, stop=True,
                )
            # ---- activations / weights in ----
            # the DMA engine drains both HWDGE queues serially, so order
            # transfers by first-use: xt tt0 + wkv (kv proj), then wq + rope
            # tables (q proj), then the rest.  r2t only feeds the rope tail
            # (the warm-up runs off a memset tile), so it loads after wkv.
            wkv_sb = wts.tile([128, 16, 128], BF)
            nc.sync.dma_start(
                wkv_sb[:], wkv[:].rearrange("(fc p) m -> p fc m", p=128)
            )
            r2t_sb = consts.tile([128, 128], BF)
            nc.sync.dma_start(r2t_sb[:], r2t[:])
            masks_sb = consts.tile([128, 128], BF)
            nc.sync.dma_start(masks_sb[:], masks[:])
            ident2_sb = consts.tile([128, 64], BF)
            nc.sync.dma_start(ident2_sb[:], ident2[:])
            cos2_sb = consts.tile([128, T], BF)
            sin2_sb = consts.tile([128, T], BF)
            xt_sb = big.tile([128, 16, T], BF, tag="big")

            # scalar HWDGE queue: xt tt0 first (kv+q proj prologue), then wq
            nc.scalar.dma_start(
                xt_sb[:, :, 0:512],
                xt[:, 0:512].rearrange("(fc p) c -> p fc c", p=128),
            )
            wq_sb = wts.tile([128, 16, 4, 128], BF)
            nc.scalar.dma_start(
                wq_sb[:, 0:8, :, :],
                wq[0:1024, :].rearrange(
                    "(fc p) (qc m) -> p fc qc m", p=128, m=128
                ),
            )
            # tables are identical in both partition halves: load 64 rows
            # and duplicate on-chip (halves the serial-DMA-engine cost)
            nc.scalar.dma_start(cos2_sb[0:64, :], cos2[:])
            nc.scalar.dma_start(sin2_sb[0:64, :], sin2[:])
            nc.sync.dma_start(cos2_sb[64:128, :], cos2_sb[0:64, :])
            nc.sync.dma_start(sin2_sb[64:128, :], sin2_sb[0:64, :])
            nc.scalar.dma_start(
                wq_sb[:, 8:16, :, :],
                wq[1024:2048, :].rearrange(
                    "(fc p) (qc m) -> p fc qc m", p=128, m=128
                ),
            )
            for fc in range(16):
                queue = nc.sync.dma_start if fc % 2 == 0 else nc.scalar.dma_start
                queue(
                    xt_sb[:, fc, 512:T],
                    xt[fc * 128 : fc * 128 + 128, 512:T],
                )
            wo_sb = wts.tile([128, 4, 16, 128], BF)
            for fcl in range(4):
                nc.scalar.dma_start(
                    wo_sb[:, fcl, :, :],
                    wo[fcl * 128 : fcl * 128 + 128, :].rearrange(
                        "p (oc m) -> p oc m", m=128
                    ),
                )

            # ---- on-chip activations ----
            kvrope_sb = acts.tile([128, T], BF)
            kdup_sb = acts.tile([128, T], BF)
            v1_sb = acts.tile([128, 16, 65], BF)
            nc.vector.memset(v1_sb[:, :, 64:65], 1.0)
            ones_sb = consts.tile([65, 64], BF)
            nc.vector.memset(ones_sb[:], 1.0)
            qrope_sb = acts.tile([128, 4, T], BF)
            ao_sb = acts.tile([128, 4, T], BF)

            scale = 1.0 / np.sqrt(HD)

            def kv_unit(tt):
                # kv projection + rope for token slice tt
                # (k rows 0..63, v rows 64..127)
                ts = slice(tt * 512, tt * 512 + 512)
                ps = psum.tile([128, 512], F32, tag="mm")
                for fc in range(16):
                    nc.tensor.matmul(
                        ps[:],
                        lhsT=wkv_sb[:, fc, :],
                        rhs=xt_sb[:, fc, ts],
                        start=(fc == 0),
                        stop=(fc == 15),
                    )
                kv_sb = work.tile([128, 512], BF, tag="evac")
                nc.vector.tensor_copy(kv_sb[:], ps[:])
                psu = psum.tile([128, 512], F32, tag="mm", name="psu")
                nc.tensor.matmul(
                    psu[0:64, :], lhsT=r2t_sb[0:64, 0:64], rhs=kv_sb[0:64, :],
                    start=True, stop=True,
                )
                t1 = work.tile([128, 512], BF, tag="t1")
                nc.vector.tensor_mul(t1[0:64, :], kv_sb[0:64, :], cos2_sb[0:64, ts])
                t2 = work.tile([128, 512], BF, tag="t2")
                nc.vector.tensor_mul(t2[0:64, :], psu[0:64, :], sin2_sb[0:64, ts])
                nc.vector.tensor_add(kvrope_sb[0:64, ts], t1[0:64, :], t2[0:64, :])
                nc.vector.tensor_copy(kvrope_sb[64:128, ts], kv_sb[64:128, :])
                # duplicate roped k into both partition halves
                nc.sync.dma_start(kdup_sb[0:64, ts], kvrope_sb[0:64, ts])
                nc.sync.dma_start(kdup_sb[64:128, ts], kvrope_sb[0:64, ts])
                # v' chunks [128 tok, 65]: col 64 = 1.0 (softmax denom trick)
                for kt in range(4 * tt, 4 * tt + 4):
                    pst = psum.tile([128, 64], BF, tag="pav", bufs=2)
                    nc.tensor.transpose(
                        pst[:],
                        kvrope_sb[64:128, kt * 128 : kt * 128 + 128],
                        ident2_sb[64:128, :],
                    )
                    nc.scalar.copy(v1_sb[:, kt, 0:64], pst[:])

            def q_unit_chunks(tt, ph):
                # q projection + rope for head pair ph, token slice tt,
                # split into filler chunks so exp/attention can overlap
                ts = slice(tt * 512, tt * 512 + 512)
                state = {}

                def acc(c0):
                    def fn():
                        if c0 == 0:
                            state["ps"] = psum.tile(
                                [128, 512], F32, tag="mm", name="psq"
                            )
                        ps = state["ps"]
                        for fc in range(c0, c0 + 4):
                            nc.tensor.matmul(
                                ps[:],
                                lhsT=wq_sb[:, fc, ph, :],
                                rhs=xt_sb[:, fc, ts],
                                start=(fc == 0),
                                stop=(fc == 15),
                            )
                    return fn

                def tail():
                    ps = state["ps"]
                    q_sb = work.tile([128, 512], BF, tag="evac")
                    nc.vector.tensor_copy(q_sb[:], ps[:])
                    psu = psum.tile([128, 512], F32, tag="mm", name="psu2")
                    nc.tensor.matmul(
                        psu[:], lhsT=r2t_sb[:], rhs=q_sb[:],
                        start=True, stop=True,
                    )
                    t1 = work.tile([128, 512], BF, tag="t1")
                    nc.vector.tensor_mul(t1[:], q_sb[:], cos2_sb[:, ts])
                    t2 = work.tile([128, 512], BF, tag="t2")
                    nc.vector.tensor_mul(t2[:], psu[:], sin2_sb[:, ts])
                    nc.vector.tensor_add(qrope_sb[:, ph, ts], t1[:], t2[:])

                return [acc(0), acc(4), acc(8), acc(12), tail]

            def emit_norm(u, last=False):
                # normalize: per-query reciprocal of the denominator row,
                # broadcast across partitions with a K=1 ones-matmul, then
                # one mul per head half straight out of PSUM.  par0 lands
                # in-lane in ao_sb; par1 needs a partition-shifting
                # SBUF->SBUF DMA.
                uph, uqb, upav = u
                uQ0 = uqb * 512
                avu = work.tile([65, 1024], BF, tag="avu")
                if last:
                    # the final unit's norm gates the wo(3) tail: skip the
                    # staging copy (early PSUM release is moot at the end)
                    # and run reciprocal straight off the accumulators; the
                    # multiplies then read pav (one PSUM operand) x bc copied
                    # through SBUF by the idle ACT engine.
                    with nc.allow_low_precision(
                        reason="bf16 softmax denominators are within tolerance"
                    ):
                        nc.vector.reciprocal(avu[64:65, 0:512], upav[0][64:65, :])
                        nc.vector.reciprocal(avu[64:65, 512:1024], upav[1][64:65, :])
                else:
                    nc.vector.tensor_copy(avu[:, 0:512], upav[0][:])
                    nc.vector.tensor_copy(avu[:, 512:1024], upav[1][:])
                    with nc.allow_low_precision(
                        reason="bf16 softmax denominators are within tolerance"
                    ):
                        nc.vector.reciprocal(avu[64:65, :], avu[64:65, :])
                bc = psum.tile([64, 1024], F32, tag="pss", bufs=2, name="bc")
                nc.tensor.matmul(
                    bc[:, 0:512], lhsT=ones_sb[64:65, :],
                    rhs=avu[64:65, 0:512], start=True, stop=True,
                )
                nc.tensor.matmul(
                    bc[:, 512:1024], lhsT=ones_sb[64:65, :],
                    rhs=avu[64:65, 512:1024], start=True, stop=True,
                )
                if last:
                    b_sb = work.tile([64, 1024], BF, tag="bcast")
                    nc.scalar.copy(b_sb[:], bc[:])
                    nc.vector.tensor_mul(
                        ao_sb[0:64, uph, uQ0 : uQ0 + 512],
                        upav[0][0:64, :], b_sb[:, 0:512],
                    )
                    av_sb = work.tile([64, 512], BF, tag="av", name="av1")
                    nc.vector.tensor_mul(
                        av_sb[:], upav[1][0:64, :], b_sb[:, 512:1024]
                    )
                else:
                    nc.vector.tensor_mul(
                        ao_sb[0:64, uph, uQ0 : uQ0 + 512],
                        bc[:, 0:512], avu[0:64, 0:512],
                    )
                    av_sb = work.tile([64, 512], BF, tag="av", name="av1")
                    nc.vector.tensor_mul(
                        av_sb[:], bc[:, 512:1024], avu[0:64, 512:1024]
                    )
                nc.sync.dma_start(
                    ao_sb[64:128, uph, uQ0 : uQ0 + 512], av_sb[:]
                )

            def attn_unit(qb, ph, pending):
                # attention for heads (2*ph, 2*ph+1), query block qb
                Q0 = qb * 512
                nkt = 4 * qb + 4
                pav = [
                    psum.tile([65, 512], F32, tag="pav", name=f"pav{i}", bufs=2)
                    for i in range(2)
                ]
                for pr in range(nkt // 2):
                    kt0, kt1 = 2 * pr, 2 * pr + 1
                    # causal-active widths (tiles above the diagonal shrink)
                    j0, j1 = kt0 - 4 * qb, kt1 - 4 * qb
                    w0 = 512 if j0 < 0 else 512 - 128 * j0
                    w1 = 512 if j1 < 0 else 512 - 128 * j1
                    diag = j0 >= 0
                    # scores for both head halves interleaved so adjacent
                    # matmuls target different PE row groups (concurrent)
                    pss = [
                        psum.tile([128, 1024], F32, tag="pss", bufs=2,
                                  name=f"pss{i}")
                        for i in range(2)
                    ]
                    for kt, w, off in ((kt0, w0, 0), (kt1, w1, w0)):
                        for par in range(2):
                            lo, hi = (0, 64) if par == 0 else (64, 128)
                            nc.tensor.matmul(
                                pss[par][:, off : off + w],
                                lhsT=kdup_sb[lo:hi, kt * 128 : kt * 128 + 128],
                                rhs=qrope_sb[lo:hi, ph, Q0 + 512 - w : Q0 + 512],
                                start=True,
                                stop=True,
                            )
                    e_pair = []
                    for par in range(2):
                        e_sb = exps.tile([128, 1024], BF, tag="e", name=f"e{par}")
                        nc.scalar.activation(
                            e_sb[:, 0 : w0 + w1], pss[par][:, 0 : w0 + w1],
                            AF.Exp, scale=scale,
                        )
                        if diag:
                            # only the first 128 query columns of each slab
                            # overlap the key tile (the tril block); the rest
                            # are fully causal-visible
                            nc.vector.tensor_mul(
                                e_sb[:, 0:128], e_sb[:, 0:128],
                                masks_sb[:, 0:128],
                            )
                            nc.vector.tensor_mul(
                                e_sb[:, w0 : w0 + 128], e_sb[:, w0 : w0 + 128],
                                masks_sb[:, 0:128],
                            )
                        e_pair.append(e_sb)
                    for kt, w, off in ((kt0, w0, 0), (kt1, w1, w0)):
                        for par in range(2):
                            nc.tensor.matmul(
                                pav[par][:, 512 - w : 512],
                                lhsT=v1_sb[:, kt, :],
                                rhs=e_pair[par][:, off : off + w],
                                start=(kt == 0),
                                stop=(kt == nkt - 1),
                            )
                    if pr == min(1, nkt // 2 - 1) and pending is not None:
                        emit_norm(pending)
                        pending = None
                    # qb3 units have 8 prs but only 4 fillers: spread them
                    if qb < 3 or pr % 2 == 1:
                        drain_filler(3 if pr == nkt // 2 - 1 else 1)
                if pending is not None:
                    emit_norm(pending)
                return (ph, qb, pav)

            # wo partial: partial[oc*128.., Q0:Q0+512] =
            #   sum_ph wo_sb[:, ph, oc, :]^T @ ao_sb[:, ph, Q0:Q0+512]
            partial = dram.tile([4, DIM, 512], BF, name="partial")

            wo_state = {}

            def wo_chunk(qb, oc, tail=False):
                # 4-oc groups share one SBUF staging tile and one store DMA.
                # Tail chunks (after the last attention unit) alternate onto
                # the freed pav ring so four ocs can prefetch their ph0-2
                # accumulation while the final norm chain resolves ph3.
                def fn():
                    Q0 = qb * 512
                    tag = "pav" if (tail and oc % 2 == 1) else "mm"
                    ps = psum.tile([128, 512], F32, tag=tag, name="pswo")
                    for ph in range(4):
                        nc.tensor.matmul(
                            ps[:],
                            lhsT=wo_sb[:, ph, oc, :],
                            rhs=ao_sb[:, ph, Q0 : Q0 + 512],
                            start=(ph == 0),
                            stop=(ph == 3),
                        )
                    if oc % 4 == 0:
                        wo_state["o_sb"] = outp.tile([128, 4, 512], BF, tag="o", name="o_sb")
                    o_sb = wo_state["o_sb"]
                    nc.vector.tensor_copy(o_sb[:, oc % 4, :], ps[:])
                    if oc % 4 == 3:
                        oc0 = oc - 3
                        nc.sync.dma_start(
                            partial[qb].rearrange(
                                "(oc p) c -> p oc c", p=128
                            )[:, oc0 : oc0 + 4, :],
                            o_sb[:],
                        )
                return fn

            filler_q = []  # entries: (tag, fn); tag = ("q", tt, ph) | ("wo", qb)

            def drain_filler(n):
                for _ in range(n):
                    if filler_q:
                        filler_q.pop(0)[1]()

            def drain_until(pred):
                # pop (FIFO) until no queued filler matches pred
                while any(pred(t) for t, _ in filler_q):
                    filler_q.pop(0)[1]()

            rsout = dram.tile([4, 512, 512], BF, name="rsout")

            def emit_rs(qb):
                nc.gpsimd.collective_compute(
                    "ReduceScatter",
                    mybir.AluOpType.add,
                    ins=[partial[qb]],
                    outs=[rsout[qb]],
                    replica_groups=GROUPS,
                )
                # the final copy rides the idle SP queue (nothing is queued
                # behind it); earlier ones stay on Pool so SP's partial
                # stores are never blocked behind a waiting collective
                if qb == 3:
                    nc.sync.dma_start(outb[qb], rsout[qb])
                else:
                    nc.gpsimd.dma_start(outb[qb], rsout[qb])

            # ---- emission schedule ----
            kv_unit(0)
            for c in q_unit_chunks(0, 0):
                c()
            for ph in range(1, 4):
                for c in q_unit_chunks(0, ph):
                    c()
            for tt in range(1, 4):
                kv_unit(tt)

            pending = None
            for qb in range(4):
                for ph in range(4):
                    if qb >= 1:
                        filler_q.extend(
                            (("wo", qb - 1), wo_chunk(qb - 1, oc))
                            for oc in range(4 * ph, 4 * ph + 4)
                        )
                    if qb < 3:
                        filler_q.extend(
                            (("q", qb + 1, ph), c)
                            for c in q_unit_chunks(qb + 1, ph)
                        )
                    # q-proj chunks for THIS unit must be emitted before the
                    # scores that read them
                    drain_until(lambda t: t[0] == "q" and t[1:] == (qb, ph))
                    pending = attn_unit(qb, ph, pending)
                if qb >= 1:
                    # wo stores for qb-1 must be emitted before its RS
                    drain_until(lambda t: t == ("wo", qb - 1))
                    emit_rs(qb - 1)
            emit_norm(pending, last=True)
            for oc in range(16):
                wo_chunk(3, oc, tail=True)()
            emit_rs(3)

    return nc


def _host_tables():
    inv_freq = 1.0 / (10000.0 ** (np.arange(0, HD, 2, dtype=np.float32) / HD))
    t = np.arange(T, dtype=np.float32)
    freqs = np.einsum("i,j->ij", t, inv_freq)
    emb = np.concatenate([freqs, freqs], axis=-1)  # [T, 64]
    cosT = np.cos(emb).T.astype(np.float32)  # [64, T]
    sinT = np.sin(emb).T.astype(np.float32)

    cos2 = np.ascontiguousarray(cosT).astype(BF16)
    sin2 = np.ascontiguousarray(sinT).astype(BF16)

    R = np.zeros((HD, HD), dtype=np.float32)
    for d in range(32):
        R[d, d + 32] = -1.0
        R[d + 32, d] = 1.0
    r2 = np.block([[R, np.zeros_like(R)], [np.zeros_like(R), R]])
    r2t = np.ascontiguousarray(r2.T).astype(BF16)  # lhsT: matmul computes R2 @ rhs

    ident2 = np.vstack([np.eye(HD), np.eye(HD)]).astype(BF16)  # [128, 64]

    r_idx = np.arange(128)[:, None]
    c_idx = np.arange(128)[None, :]
    masks = (c_idx >= r_idx).astype(np.float32).astype(BF16)  # [128, 128]

    return dict(
        cos2=cos2, sin2=sin2, r2t=r2t, ident2=ident2, masks=masks,
    )


_STATE = {}


def _get_nc():
    if "nc" not in _STATE:
        _STATE["nc"] = _build_nc()
        _STATE["tables"] = _host_tables()
    return _STATE["nc"], _STATE["tables"]


def _make_in_maps(x, wq, wk, wv, wo, tables):
    x = np.asarray(x, dtype=np.float32)
    wq_b = np.asarray(wq, dtype=np.float32).astype(BF16)
    wo_b = np.asarray(wo, dtype=np.float32).astype(BF16)
    wk_b = np.asarray(wk, dtype=np.float32).astype(BF16)
    wv_b = np.asarray(wv, dtype=np.float32).astype(BF16)

    in_maps = []
    xt_b = [np.ascontiguousarray(x[b].T).astype(BF16) for b in range(2)]
    for core in range(N_CORES):
        b, g = core // 4, core % 4
        m = dict(tables)
        m["xt"] = xt_b[b]
        m["wq"] = np.ascontiguousarray(wq_b[:, 512 * g : 512 * g + 512])
        m["wkv"] = np.ascontiguousarray(
            np.concatenate(
                [wk_b[:, 64 * g : 64 * g + 64], wv_b[:, 64 * g : 64 * g + 64]],
                axis=1,
            )
        )
        m["wo"] = np.ascontiguousarray(wo_b[512 * g : 512 * g + 512, :])
        in_maps.append(m)
    return in_maps


def _assemble(results):
    out = np.empty((2, T, DIM), dtype=np.float32)
    for core in range(N_CORES):
        b, g = core // 4, core % 4
        ob = results[core]["outb"]  # [4 qb, 512 outfeat, 512 tok]
        out[b][:, 512 * g : 512 * g + 512] = (
            np.concatenate(list(ob), axis=1).T.astype(np.float32)
        )
    return out


def kernel(x, wq, wk, wv, wo):
    nc, tables = _get_nc()
    in_maps = _make_in_maps(x, wq, wk, wv, wo, tables)
    res = run_bass_kernel_spmd(
        nc, in_maps, core_ids=list(range(N_CORES)), trace=False
    )
    return _assemble(res.results)
